# revision 1
# baseline (speedup 1.0000x reference)
"""Trainium (trn2) Bass kernel for a 2-layer GAT over N=100k nodes / E=1.7M edges.

Strategy
--------
Edges are sorted by destination on the host (index-only preprocessing); the
destination axis is sharded across the 8 NeuronCores in contiguous 128-node
"windows" (98 windows per core). All floating-point work runs on device:

* per-edge source features h_e = x[src] @ W arrive as a matmul over
  host-permuted, transposed fp16 input streams (the host only permutes /
  casts inputs - it performs no FLOPs);
* attention logits z = a_s.h[src] + a_d.h[dst]: the source term comes out of
  the same matmul via folded weight columns; the destination term is expanded
  from a per-window node table via a streamed one-hot S^T matmul that
  accumulates directly into the logit columns of the same PSUM slot;
* leaky_relu+exp run batched on the Scalar engine (Prelu/Exp share one
  activation table; a constant -4 bias inside Exp keeps fp16 exp in range and
  cancels in the softmax);
* messages m = h_e * exp(z) (Vector engine, head-broadcast access pattern)
  are segment-summed per 128-node window by a matmul with an on-chip
  selection matrix S[e, n] = (rel_dst[e] == n) built with one
  tensor_scalar(is_equal); denominators ride along as extra columns of the
  same PSUM accumulation, so the softmax division happens once per window.

Layer 1 (8 heads x 16) + ELU and layer 2 (1 head x 64) are two SPMD kernels
with a host permutation of the layer-1 output in between. The NEFF is
specialized to the edge distribution (per-window tile counts) and cached.

Environment workarounds: this container's walrus build allows only ONE
semaphore wait per instruction (split onto nop carriers post-scheduling), and
the GPSIMD ucode libraries are absent (so no dma_gather/indirect-DMA fast
paths - hence the streamed-matmul design).
"""
import sys
import os
import time

import numpy as np

import concourse.bass as bass
import concourse.mybir as mybir
import concourse.tile as tile
from concourse.bass_utils import run_bass_kernel_spmd

P = 128
F16 = mybir.dt.float16
F32 = mybir.dt.float32
AF = mybir.ActivationFunctionType
OP = mybir.AluOpType
NEG_SLOPE = 0.2
EXP_BIAS = -4.0     # exp(z + EXP_BIAS): constant shift cancels in softmax
GRP = 16            # tiles per stream group
PAD_REL = 255.0     # rel value for pad slots -> is_equal never matches
N_CORES = 8

# ------------------------------------------------------------------ patches

_wsplit_counter = [0]


def _split_excess_waits(nc, max_waits=1):
    """This walrus build rejects >1 sem-wait per instruction ("Too many sync
    wait commands"). Move overflow waits onto same-engine nop carriers."""
    n_split = 0
    for f in nc.m.functions:
        for blk in f.blocks:
            changed = False
            out = []
            for inst in blk.instructions:
                si = inst.sync_info
                if si is not None and len(si.on_wait) > max_waits:
                    waits = list(si.on_wait)
                    keep = waits[len(waits) - max_waits:]
                    overflow = waits[: len(waits) - max_waits]
                    for i in range(0, len(overflow), max_waits):
                        _wsplit_counter[0] += 1
                        nop = mybir.InstNoOp(
                            name=f"I-wsplit-{_wsplit_counter[0]}", ins=[], outs=[])
                        nop.engine = inst.engine
                        nop.sync_info = mybir.SyncInfo(
                            on_wait=overflow[i: i + max_waits], on_update=[])
                        out.append(nop)
                    inst.sync_info = mybir.SyncInfo(
                        on_wait=keep, on_update=list(si.on_update))
                    changed = True
                    n_split += 1
                out.append(inst)
            if changed:
                blk.instructions = out
    return n_split


def _finalize_kernel(nc):
    import bass_rust as _bass_rust
    from concourse.library_config import all_libraries, standard
    from concourse.library_overlay import lower_extended_insts

    inst_type_to_lib_mask = {}
    for lib in all_libraries:
        for inst_type in lib.instructions:
            inst_type_to_lib_mask[inst_type] = inst_type_to_lib_mask.get(
                inst_type, 0) | (1 << lib.index)
    _bass_rust.insert_library_loads(
        nc, inst_type_to_lib_mask, len(all_libraries), standard.index)
    lower_extended_insts(nc)
    _split_excess_waits(nc)


# ------------------------------------------------------------------ host prep

class _Graph:
    """Host-side index preprocessing: sort by dst, shard dst windows across
    cores, pad per-window tile counts to a global schedule so all cores run
    one identical SPMD program."""

    def __init__(self, edge_index, n_nodes, n_cores):
        self.N = n_nodes
        self.C = n_cores
        src = np.asarray(edge_index[0], dtype=np.int64)
        dst = np.asarray(edge_index[1], dtype=np.int64)
        perm = np.argsort(dst, kind="stable")
        self.src_s = src[perm].astype(np.int32)
        self.dst_s = dst[perm].astype(np.int32)

        n_win_total = (n_nodes + P - 1) // P
        self.wpc = (n_win_total + n_cores - 1) // n_cores
        self.n_win = self.wpc * n_cores
        self.shard_nodes = self.wpc * P

        bounds = np.searchsorted(self.dst_s, np.arange(0, self.n_win + 1) * P)
        counts = np.zeros((n_cores, self.wpc), dtype=np.int64)
        for k in range(n_cores):
            for i in range(self.wpc):
                w = k * self.wpc + i
                if w < n_win_total:
                    counts[k, i] = bounds[w + 1] - bounds[w]
        self.PC = np.maximum(np.ceil(counts / P).astype(np.int64).max(axis=0), 1)
        self.T = int(self.PC.sum())

        self.slot_src = np.zeros((n_cores, self.T * P), dtype=np.int32)
        self.slot_rel = np.full((n_cores, self.T * P), int(PAD_REL), dtype=np.int32)
        for k in range(n_cores):
            t0 = 0
            for i in range(self.wpc):
                w = k * self.wpc + i
                cnt = int(counts[k, i])
                if cnt > 0:
                    e0 = bounds[w]
                    sl = t0 * P
                    self.slot_src[k, sl:sl + cnt] = self.src_s[e0:e0 + cnt]
                    self.slot_rel[k, sl:sl + cnt] = self.dst_s[e0:e0 + cnt] - w * P
                t0 += int(self.PC[i])

        # graph-only streams, cached per core
        self._st = {}
        self._rel = {}

    def stream_srcT(self, table_T_f16, core):
        return np.ascontiguousarray(table_T_f16[:, self.slot_src[core]])

    def stream_ST(self, core):
        if core not in self._st:
            out = np.zeros((P, self.T * P), dtype=np.float16)
            rel = self.slot_rel[core]
            valid = rel < P
            out[rel[valid], np.nonzero(valid)[0]] = 1.0
            self._st[core] = out
        return self._st[core]

    def stream_rel(self, core):
        if core not in self._rel:
            self._rel[core] = np.ascontiguousarray(
                self.slot_rel[core].reshape(self.T, P).T.astype(np.float32))
        return self._rel[core]


# ------------------------------------------------------------------ builder

def _build_layer(T, PC, wpc, c_in, heads, hid, elu, add_bias, bench_loop=1):
    HC = heads * hid
    CA = HC + heads
    SLOT = HC + heads
    n_slots = max(1, min(2048 // (SLOT * 4), GRP))

    nc = bass.Bass()
    xT = nc.dram_tensor("xT", [c_in, wpc * P], F16, kind="ExternalInput")
    xsrcT = nc.dram_tensor("xsrcT", [c_in, T * P], F16, kind="ExternalInput")
    stT = nc.dram_tensor("stT", [P, T * P], F16, kind="ExternalInput")
    rel = nc.dram_tensor("rel", [P, T], F32, kind="ExternalInput")
    iota_c = nc.dram_tensor("iota", [P, P], F16, kind="ExternalInput")
    wext = nc.dram_tensor("wext", [c_in, CA], F16, kind="ExternalInput")
    wald = nc.dram_tensor("wald", [c_in, heads], F16, kind="ExternalInput")
    if add_bias:
        brep = nc.dram_tensor("brep", [P, HC], F32, kind="ExternalInput")
    out = nc.dram_tensor("out", [wpc * P, HC], F32, kind="ExternalOutput")

    n_groups = (T + GRP - 1) // GRP

    with tile.TileContext(nc) as tc:
        with (
            tc.tile_pool(name="const", bufs=1) as constp,
            tc.tile_pool(name="ald", bufs=1) as aldp,
            tc.tile_pool(name="stream", bufs=3) as streamp,
            tc.tile_pool(name="work", bufs=4) as workp,
            tc.tile_pool(name="msg", bufs=4) as msgp,
            tc.tile_pool(name="epi", bufs=3) as epip,
            tc.tile_pool(name="psA", bufs=3, space="PSUM") as psA,
            tc.tile_pool(name="psW", bufs=2, space="PSUM") as psW,
            tc.tile_pool(name="psN", bufs=2, space="PSUM") as psN,
        ):
            iota_sb = constp.tile([P, P], F16)
            nc.sync.dma_start(out=iota_sb[:], in_=iota_c[:])
            wext_sb = constp.tile([c_in, CA], F16)
            nc.sync.dma_start(out=wext_sb[:], in_=wext[:])
            wald_sb = constp.tile([c_in, heads], F16)
            nc.sync.dma_start(out=wald_sb[:], in_=wald[:])
            if add_bias:
                brep_sb = constp.tile([P, HC], F32)
                nc.sync.dma_start(out=brep_sb[:], in_=brep[:])
            ebias_sb = constp.tile([P, 1], F32)
            nc.vector.memset(ebias_sb[:], EXP_BIAS)

            # node phase: per-shard alD table, SBUF-resident [P, wpc, heads]
            ald_tab = aldp.tile([P, wpc, heads], F16)
            NB = 3
            for c0 in range(0, wpc, NB):
                nb = min(NB, wpc - c0)
                ps = psN.tile([P, NB * heads], F32, tag="psN")
                xc = workp.tile([c_in, NB * P], F16, tag="xc")
                nc.sync.dma_start(out=xc[:, :nb * P],
                                  in_=xT[:, c0 * P:(c0 + nb) * P])
                for c in range(nb):
                    nc.tensor.matmul(
                        ps[:, c * heads:(c + 1) * heads],
                        xc[:, c * P:(c + 1) * P], wald_sb[:],
                        start=True, stop=True)
                nc.vector.tensor_copy(
                    ald_tab[:, c0:c0 + nb, :],
                    ps[:].rearrange("p (c h) -> p c h", c=NB)[:, :nb, :])

            # edge phase
            tile_win = []
            for i in range(wpc):
                tile_win += [i] * int(PC[i])
            first_of_win, last_of_win = {}, {}
            for t, w in enumerate(tile_win):
                first_of_win.setdefault(w, t)
                last_of_win[w] = t

            def edge_phase(_iv=None):
                psw_cur = None
                for g in range(n_groups):
                    tlo, thi = g * GRP, min(T, g * GRP + GRP)
                    ng = thi - tlo
                    xs_g = streamp.tile([c_in, GRP * P], F16, tag="xs")
                    nc.sync.dma_start(out=xs_g[:, :ng * P],
                                      in_=xsrcT[:, tlo * P:thi * P])
                    st_g = streamp.tile([P, GRP * P], F16, tag="st")
                    nc.sync.dma_start(out=st_g[:, :ng * P],
                                      in_=stT[:, tlo * P:thi * P])
                    rel_g = streamp.tile([P, GRP], F32, tag="rel")
                    nc.sync.dma_start(out=rel_g[:, :ng], in_=rel[:, tlo:thi])

                    for s0 in range(tlo, thi, n_slots):
                        s1 = min(thi, s0 + n_slots)
                        ns = s1 - s0
                        psa = psA.tile([P, n_slots * SLOT], F32, tag="psA")
                        for j, t in enumerate(range(s0, s1)):
                            w = tile_win[t]
                            col = (t - tlo) * P
                            nc.tensor.matmul(
                                psa[:, j * SLOT:j * SLOT + CA],
                                xs_g[:, col:col + P], wext_sb[:],
                                start=True, stop=False)
                            nc.tensor.matmul(
                                psa[:, j * SLOT + HC:j * SLOT + HC + heads],
                                st_g[:, col:col + P], ald_tab[:, w, :],
                                start=False, stop=True)
                        zsl = psa[:].rearrange(
                            "p (s f) -> p s f", s=n_slots)[:, :ns, HC:HC + heads]
                        nc.scalar.activation(zsl, zsl, AF.Prelu, alpha=NEG_SLOPE)
                        exp_sb = workp.tile([P, n_slots, heads], F16, tag="exp")
                        nc.scalar.activation(exp_sb[:, :ns, :], zsl, AF.Exp,
                                             bias=ebias_sb[:])

                        for j, t in enumerate(range(s0, s1)):
                            w = tile_win[t]
                            col_t = t - tlo
                            S_sb = workp.tile([P, P], F16, tag="S")
                            nc.any.tensor_scalar(
                                S_sb[:], iota_sb[:],
                                rel_g[:, col_t:col_t + 1], None, OP.is_equal)
                            msg_sb = msgp.tile([P, HC], F16, tag="msg")
                            e_ap = exp_sb[:, j, :]
                            e_b = bass.AP(e_ap.tensor, e_ap.offset,
                                          [e_ap.ap[0], [1, heads], [0, hid]])
                            nc.any.tensor_tensor(
                                out=msg_sb[:],
                                in0=psa[:, j * SLOT:j * SLOT + HC],
                                in1=e_b, op=OP.mult)
                            if t == first_of_win[w]:
                                psw_cur = psW.tile([P, HC + heads], F32, tag="psW")
                            nc.tensor.matmul(
                                psw_cur[:, 0:HC], S_sb[:], msg_sb[:],
                                start=(t == first_of_win[w]), stop=False)
                            nc.tensor.matmul(
                                psw_cur[:, HC:HC + heads], S_sb[:],
                                exp_sb[:, j, :],
                                start=False, stop=(t == last_of_win[w]))
                            if t == last_of_win[w]:
                                _epilogue(nc, epip, psw_cur, w, out, heads,
                                          hid, elu, add_bias,
                                          brep_sb if add_bias else None)

            if bench_loop > 1:
                with tc.For_i(0, bench_loop, 1) as _iv:
                    edge_phase(_iv)
            else:
                edge_phase()
    _finalize_kernel(nc)
    return nc


def _epilogue(nc, epip, psw, w, out, heads, hid, elu, add_bias, brep_sb):
    HC = heads * hid
    den = epip.tile([P, heads], F32, tag="den")
    nc.vector.tensor_scalar(den[:], psw[:, HC:HC + heads], 1e-30, None, OP.add)
    rec = epip.tile([P, heads], F32, tag="rec")
    nc.vector.reciprocal(rec[:], den[:])
    r_ap = rec[:]
    r_b = bass.AP(r_ap.tensor, r_ap.offset, [r_ap.ap[0], [1, heads], [0, hid]])
    o1 = epip.tile([P, HC], F32, tag="o1")
    nc.vector.tensor_tensor(out=o1[:], in0=psw[:, 0:HC], in1=r_b, op=OP.mult)
    if add_bias:
        nc.vector.tensor_tensor(out=o1[:], in0=o1[:], in1=brep_sb[:], op=OP.add)
    if elu:
        mn = epip.tile([P, HC], F32, tag="mn")
        nc.vector.tensor_scalar(mn[:], o1[:], 0.0, None, OP.min)
        ex = epip.tile([P, HC], F32, tag="ex")
        nc.scalar.activation(ex[:], mn[:], AF.Exp)
        mx = epip.tile([P, HC], F32, tag="mx")
        nc.vector.tensor_scalar(mx[:], o1[:], 0.0, -1.0, OP.max, OP.add)
        nc.vector.tensor_tensor(out=mx[:], in0=mx[:], in1=ex[:], op=OP.add)
        res = mx
    else:
        res = o1
    nc.sync.dma_start(out=out[w * P:(w + 1) * P, :], in_=res[:])


# ------------------------------------------------------------------ runner

def _fold_att(W, a):
    heads, hid = a.shape
    return np.einsum("ihc,hc->ih", W.reshape(W.shape[0], heads, hid), a)


class _GatRunner:
    def __init__(self, n_cores=N_CORES):
        self.C = n_cores
        self._graph = None
        self._graph_key = None
        self._kernels = {}

    def graph(self, edge_index, n_nodes):
        key = hash(np.asarray(edge_index).tobytes())
        if key != self._graph_key:
            self._graph = _Graph(edge_index, n_nodes, self.C)
            self._graph_key = key
            self._kernels.clear()
        return self._graph

    def kernel(self, name, g, c_in, heads, hid, elu, add_bias, bench_loop=1):
        key = (name, g.T, c_in, heads, hid, elu, add_bias, bench_loop)
        if key not in self._kernels:
            self._kernels[key] = _build_layer(
                g.T, g.PC, g.wpc, c_in, heads, hid, elu, add_bias, bench_loop)
        return self._kernels[key]

    def layer_inputs(self, g, table_T_f16, W, a_s, a_d, b, heads, hid):
        wextv = np.concatenate([W, _fold_att(W, a_s)], axis=1).astype(np.float16)
        waldv = _fold_att(W, a_d).astype(np.float16)
        iota_v = np.tile(np.arange(P, dtype=np.float16), (P, 1))
        bnz = bool(np.any(b))
        maps = []
        for k in range(self.C):
            im = {
                "xT": np.ascontiguousarray(
                    table_T_f16[:, k * g.shard_nodes:(k + 1) * g.shard_nodes]),
                "xsrcT": g.stream_srcT(table_T_f16, k),
                "stT": g.stream_ST(k),
                "rel": g.stream_rel(k),
                "iota": iota_v,
                "wext": wextv,
                "wald": waldv,
            }
            if bnz:
                im["brep"] = np.tile(np.asarray(b, np.float32), (P, 1))
            maps.append(im)
        return maps, bnz

    def run(self, x, edge_index, W1, a_src1, a_dst1, b1, W2, a_src2, a_dst2, b2,
            bench_loop=1):
        C = self.C
        N, IN_C = x.shape
        HEADS, HID = a_src1.shape
        HC = HEADS * HID
        OUT_C = W2.shape[1]
        g = self.graph(edge_index, N)

        xT_pad = np.zeros((IN_C, g.n_win * P), dtype=np.float16)
        xT_pad[:, :N] = np.asarray(x, np.float32).T
        mapsA, b1nz = self.layer_inputs(g, xT_pad, W1, a_src1, a_dst1, b1,
                                        HEADS, HID)
        ncA = self.kernel("A", g, IN_C, HEADS, HID, True, b1nz, bench_loop)
        resA = run_bass_kernel_spmd(ncA, mapsA, core_ids=list(range(C)))
        out1 = np.concatenate([r["out"] for r in resA.results], axis=0)

        o1T_pad = np.zeros((HC, g.n_win * P), dtype=np.float16)
        o1T_pad[:, :N] = out1[:N].T
        mapsB, b2nz = self.layer_inputs(g, o1T_pad, W2, a_src2, a_dst2, b2,
                                        1, OUT_C)
        ncB = self.kernel("B", g, HC, 1, OUT_C, False, b2nz, bench_loop)
        resB = run_bass_kernel_spmd(ncB, mapsB, core_ids=list(range(C)))
        return np.concatenate([r["out"] for r in resB.results], axis=0)[:N]


_RUNNER = _GatRunner()


def kernel(x, edge_index, W1, a_src1, a_dst1, b1, W2, a_src2, a_dst2, b2):
    """Full-input / full-output entry point. Returns [N, OUT_C] float32."""
    args = [np.asarray(v) for v in
            (x, edge_index, W1, a_src1, a_dst1, b1, W2, a_src2, a_dst2, b2)]
    return _RUNNER.run(*args).astype(np.float32)



# revision 11
# speedup vs baseline: 12.0355x; 12.0355x over previous
"""Trainium (trn2) Bass kernel for a 2-layer GAT over N=100k nodes / E=1.7M edges.

Strategy (v2 — gather-streamed edge phase)
------------------------------------------
Edges are sorted by destination on the host (index-only preprocessing); the
destination axis is sharded across the 8 NeuronCores in contiguous 128-node
windows (98 per core).  Three SPMD kernels per forward pass:

* N1 (node phase): H1ext = x @ [W1 | W1.a_src | W1.a_dst]  -> [N, 144] f16
  table, node windows sharded across cores.
* host (permutation/cast only, no FLOPs): gather H1ext rows by edge source
  (h + a_src.h) and by edge destination (a_dst.h), pack them together with
  the relative-destination column into a DMA-friendly per-core stream laid
  out [128 partitions][T tiles, C cols] so every partition reads long
  contiguous runs.
* E1 (edge phase L1): per 128-edge tile: z = als+ald; Prelu; one batched
  Exp expanded to all 128 message columns (so the message multiply is an
  all-SBUF packed-f16 TensorTensor in 2x DVE mode); an on-chip one-hot
  S[e,n]=(rel==n) built with tensor_scalar(is_equal); segment-sum +
  softmax denominators via two PSUM-accumulating matmuls per tile.  The
  per-window epilogue divides by the denominator, applies ELU, and fuses
  layer 2's node matmul (PE transpose + o2 @ [W2 | W2.a_src2 | W2.a_dst2])
  so E2 only needs 66-column gathers.
* E2 (edge phase L2): same structure with 1 head / 64 channels; outputs the
  final [N, 64] f32.

All floating-point work runs on device; the host only sorts/gathers/casts.
The NEFF is specialized to the edge distribution and cached.

Environment workarounds: this walrus build allows only ONE semaphore wait
per instruction (split onto nop carriers post-scheduling), and the GPSIMD
ucode libraries are absent (no dma_gather/indirect-DMA fast paths - hence
the host-gathered streams).
"""
import sys
import os
import time

import numpy as np

import concourse.bass as bass
import concourse.mybir as mybir
import concourse.tile as tile
from concourse.bass_utils import run_bass_kernel_spmd

P = 128
F16 = mybir.dt.float16
F32 = mybir.dt.float32
AF = mybir.ActivationFunctionType
OP = mybir.AluOpType
NEG_SLOPE = 0.2
EXP_BIAS = -4.0     # exp(z + EXP_BIAS): constant shift cancels in softmax
GRP = 32            # tiles per stream group
PAD_REL = 255.0     # rel value for pad slots -> is_equal never matches
N_CORES = 8

# engine-assignment tuning knobs (read at kernel-build time)
CFG = {
    "pool_s_every": 0,    # every k-th S-build on Pool engine (0 = never)
    "z_add_pool": False,  # z = als+ald on Pool instead of DVE
    "epi_pool": False,    # ELU min/max + den-eps on Pool
    "copy_act": False,    # o2T/h2/out copies on ACT (activation Copy)
}

# ------------------------------------------------------------------ patches

_wsplit_counter = [0]


def _split_excess_waits(nc, max_waits=1):
    """This walrus build rejects >1 sem-wait per instruction ("Too many sync
    wait commands"). Move overflow waits onto same-engine nop carriers."""
    n_split = 0
    for f in nc.m.functions:
        for blk in f.blocks:
            changed = False
            out = []
            for inst in blk.instructions:
                si = inst.sync_info
                if si is not None and len(si.on_wait) > max_waits:
                    waits = list(si.on_wait)
                    keep = waits[len(waits) - max_waits:]
                    overflow = waits[: len(waits) - max_waits]
                    for i in range(0, len(overflow), max_waits):
                        _wsplit_counter[0] += 1
                        nop = mybir.InstNoOp(
                            name=f"I-wsplit-{_wsplit_counter[0]}", ins=[], outs=[])
                        nop.engine = inst.engine
                        nop.sync_info = mybir.SyncInfo(
                            on_wait=overflow[i: i + max_waits], on_update=[])
                        out.append(nop)
                    inst.sync_info = mybir.SyncInfo(
                        on_wait=keep, on_update=list(si.on_update))
                    changed = True
                    n_split += 1
                out.append(inst)
            if changed:
                blk.instructions = out
    return n_split


def _finalize_kernel(nc):
    import bass_rust as _bass_rust
    from concourse.library_config import all_libraries, standard
    from concourse.library_overlay import lower_extended_insts

    inst_type_to_lib_mask = {}
    for lib in all_libraries:
        for inst_type in lib.instructions:
            inst_type_to_lib_mask[inst_type] = inst_type_to_lib_mask.get(
                inst_type, 0) | (1 << lib.index)
    _bass_rust.insert_library_loads(
        nc, inst_type_to_lib_mask, len(all_libraries), standard.index)
    lower_extended_insts(nc)
    _split_excess_waits(nc)


def _bc(ap, *dims):
    """Append free dims [stride, size] to an AP (for broadcast/stride views)."""
    return bass.AP(ap.tensor, ap.offset, list(ap.ap) + [list(d) for d in dims])


# ------------------------------------------------------------------ host prep

class _Graph:
    """Host-side index preprocessing: sort by dst, shard dst windows across
    cores, pad per-window tile counts to a global schedule so all cores run
    one identical SPMD program."""

    def __init__(self, edge_index, n_nodes, n_cores):
        self.N = n_nodes
        self.C = n_cores
        src = np.asarray(edge_index[0], dtype=np.int64)
        dst = np.asarray(edge_index[1], dtype=np.int64)
        perm = np.argsort(dst, kind="stable")
        self.src_s = src[perm].astype(np.int32)
        self.dst_s = dst[perm].astype(np.int32)

        n_win_total = (n_nodes + P - 1) // P
        self.wpc = (n_win_total + n_cores - 1) // n_cores
        self.n_win = self.wpc * n_cores
        self.shard_nodes = self.wpc * P

        bounds = np.searchsorted(self.dst_s, np.arange(0, self.n_win + 1) * P)
        counts = np.zeros((n_cores, self.wpc), dtype=np.int64)
        for k in range(n_cores):
            for i in range(self.wpc):
                w = k * self.wpc + i
                if w < n_win_total:
                    counts[k, i] = bounds[w + 1] - bounds[w]
        self.PC = np.maximum(np.ceil(counts / P).astype(np.int64).max(axis=0), 1)
        self.T = int(self.PC.sum())

        self.slot_src = np.zeros((n_cores, self.T * P), dtype=np.int32)
        self.slot_dst = np.zeros((n_cores, self.T * P), dtype=np.int32)
        self.slot_rel = np.full((n_cores, self.T * P), int(PAD_REL), dtype=np.int32)
        for k in range(n_cores):
            t0 = 0
            for i in range(self.wpc):
                w = k * self.wpc + i
                cnt = int(counts[k, i])
                if cnt > 0:
                    e0 = bounds[w]
                    sl = t0 * P
                    self.slot_src[k, sl:sl + cnt] = self.src_s[e0:e0 + cnt]
                    self.slot_dst[k, sl:sl + cnt] = self.dst_s[e0:e0 + cnt]
                    self.slot_rel[k, sl:sl + cnt] = self.dst_s[e0:e0 + cnt] - w * P
                t0 += int(self.PC[i])
        # rel as [P, T] f16 (col t = rel of edges t*P .. t*P+127)
        self.rel_pt = np.ascontiguousarray(
            self.slot_rel.reshape(n_cores, self.T, P).transpose(0, 2, 1)
        ).astype(np.float16)

    def build_stream(self, table, core, n_src_cols, n_dst_cols):
        """Pack per-edge gathered rows into the DMA-friendly stream layout
        [128][T, C] where C = n_src_cols + n_dst_cols + 1 (rel).  The table
        is [n_win*P, Ctab] f16; cols [0:n_src_cols] are gathered by edge
        source, cols [n_src_cols:n_src_cols+n_dst_cols] by edge dest."""
        T, C = self.T, n_src_cols + n_dst_cols + 1
        out = np.empty((P, T, C), dtype=np.float16)
        gs = table[self.slot_src[core], :n_src_cols]
        out[:, :, :n_src_cols] = gs.reshape(T, P, n_src_cols).transpose(1, 0, 2)
        gd = table[self.slot_dst[core], n_src_cols:n_src_cols + n_dst_cols]
        out[:, :, n_src_cols:n_src_cols + n_dst_cols] = (
            gd.reshape(T, P, n_dst_cols).transpose(1, 0, 2))
        out[:, :, C - 1] = self.rel_pt[core]
        return np.ascontiguousarray(out).reshape(P, T * C)


# ------------------------------------------------------------------ builders

def _build_node_kernel(wpc, c_in, c_out, bench_loop=1):
    """H = xT.T @ wext per node window; out [wpc*P, c_out] f16."""
    nc = bass.Bass()
    xT = nc.dram_tensor("xT", [c_in, wpc * P], F16, kind="ExternalInput")
    wext = nc.dram_tensor("wext", [c_in, c_out], F16, kind="ExternalInput")
    out = nc.dram_tensor("out", [wpc * P, c_out], F16, kind="ExternalOutput")

    NB = 3
    with tile.TileContext(nc) as tc:
        with (
            tc.tile_pool(name="const", bufs=1) as constp,
            tc.tile_pool(name="x", bufs=3) as xp,
            tc.tile_pool(name="o", bufs=3) as op_,
            tc.tile_pool(name="ps", bufs=2, space="PSUM") as psp,
        ):
            wext_sb = constp.tile([c_in, c_out], F16)
            nc.sync.dma_start(out=wext_sb[:], in_=wext[:])

            def node_phase(_iv=None):
                for c0 in range(0, wpc, NB):
                    nb = min(NB, wpc - c0)
                    xc = xp.tile([c_in, NB * P], F16, tag="xc")
                    nc.sync.dma_start(out=xc[:, :nb * P],
                                      in_=xT[:, c0 * P:(c0 + nb) * P])
                    ps = psp.tile([P, NB * c_out], F32, tag="ps")
                    for c in range(nb):
                        nc.tensor.matmul(
                            ps[:, c * c_out:(c + 1) * c_out],
                            xc[:, c * P:(c + 1) * P], wext_sb[:],
                            start=True, stop=True)
                    ot = op_.tile([P, NB * c_out], F16, tag="ot")
                    nc.vector.tensor_copy(ot[:, :nb * c_out], ps[:, :nb * c_out])
                    dst = out[c0 * P:(c0 + nb) * P, :].rearrange(
                        "(c p) f -> p c f", p=P)
                    nc.sync.dma_start(
                        out=dst,
                        in_=ot[:, :nb * c_out].rearrange(
                            "p (c f) -> p c f", c=nb))

            if bench_loop > 1:
                with tc.For_i(0, bench_loop, 1) as _iv:
                    node_phase(_iv)
            else:
                node_phase()
    _finalize_kernel(nc)
    return nc


def _build_edge_kernel(T, PC, wpc, heads, hid, elu, add_bias, fuse_cols,
                       bench_loop=1):
    """Edge phase. Stream cols: [h (HC) | als (heads) | ald (heads) | rel].
    If fuse_cols > 0: epilogue computes o2 @ w2ext -> out [wpc*P, fuse_cols]
    f16 (layer-2 node phase fused in).  Else out is [wpc*P, HC] f32."""
    HC = heads * hid
    C = HC + 2 * heads + 1
    nc = bass.Bass()
    xs = nc.dram_tensor("xs", [P, T * C], F16, kind="ExternalInput")
    iota_c = nc.dram_tensor("iota", [P, P], F16, kind="ExternalInput")
    if fuse_cols:
        ident_c = nc.dram_tensor("ident", [P, P], F16, kind="ExternalInput")
        w2ext = nc.dram_tensor("w2ext", [P, fuse_cols], F16, kind="ExternalInput")
        out = nc.dram_tensor("out", [wpc * P, fuse_cols], F16,
                             kind="ExternalOutput")
    else:
        out = nc.dram_tensor("out", [wpc * P, HC], F32, kind="ExternalOutput")
    if add_bias:
        brep = nc.dram_tensor("brep", [P, HC], F32, kind="ExternalInput")

    n_groups = (T + GRP - 1) // GRP
    tile_win = []
    for i in range(wpc):
        tile_win += [i] * int(PC[i])
    first_of_win, last_of_win = {}, {}
    for t, w in enumerate(tile_win):
        first_of_win.setdefault(w, t)
        last_of_win[w] = t

    with tile.TileContext(nc) as tc:
        with (
            tc.tile_pool(name="const", bufs=1) as constp,
            tc.tile_pool(name="stream", bufs=3) as streamp,
            tc.tile_pool(name="smat", bufs=2) as sp_,
            tc.tile_pool(name="msg", bufs=2) as msgp,
            tc.tile_pool(name="zexp", bufs=2) as zp,
            tc.tile_pool(name="epi", bufs=2) as epip,
            tc.tile_pool(name="psW", bufs=4, space="PSUM") as psW,
            tc.tile_pool(name="psE", bufs=2, space="PSUM") as psE,
        ):
            iota_sb = constp.tile([P, P], F16)
            nc.sync.dma_start(out=iota_sb[:], in_=iota_c[:])
            if fuse_cols:
                ident_sb = constp.tile([P, P], F16)
                nc.sync.dma_start(out=ident_sb[:], in_=ident_c[:])
                w2_sb = constp.tile([P, fuse_cols], F16)
                nc.sync.dma_start(out=w2_sb[:], in_=w2ext[:])
            if add_bias:
                brep_sb = constp.tile([P, HC], F32)
                nc.sync.dma_start(out=brep_sb[:], in_=brep[:])
            ebias_sb = constp.tile([P, 1], F32)
            nc.vector.memset(ebias_sb[:], EXP_BIAS)

            def epilogue(w, psw):
                epi_eng = nc.gpsimd if CFG["epi_pool"] else nc.vector
                den = epip.tile([P, heads], F32, tag="den")
                epi_eng.tensor_scalar(den[:], psw[:, HC:HC + heads],
                                      1e-30, None, OP.add)
                rec = epip.tile([P, heads], F32, tag="rec")
                nc.vector.reciprocal(rec[:], den[:])
                o1 = epip.tile([P, HC], F32, tag="o1")
                r_ap = rec[:]
                nc.vector.tensor_tensor(
                    out=o1[:], in0=psw[:, 0:HC],
                    in1=bass.AP(r_ap.tensor, r_ap.offset,
                                [r_ap.ap[0], [1, heads], [0, hid]]),
                    op=OP.mult)
                if add_bias:
                    nc.vector.tensor_tensor(out=o1[:], in0=o1[:],
                                            in1=brep_sb[:], op=OP.add)
                if elu:
                    mn = epip.tile([P, HC], F32, tag="mn")
                    epi_eng.tensor_scalar(mn[:], o1[:], 0.0, None, OP.min)
                    ex = epip.tile([P, HC], F32, tag="ex")
                    nc.scalar.activation(ex[:], mn[:], AF.Exp)
                    mx = epip.tile([P, HC], F32, tag="mx")
                    epi_eng.tensor_scalar(mx[:], o1[:], 0.0, -1.0,
                                          OP.max, OP.add)
                else:
                    ex = mx = None
                if fuse_cols:
                    o2 = epip.tile([P, HC], F16, tag="o2")
                    if elu:
                        nc.vector.tensor_tensor(out=o2[:], in0=mx[:], in1=ex[:],
                                                op=OP.add)
                    else:
                        nc.vector.tensor_copy(o2[:], o1[:])
                    psT = psE.tile([P, P], F16, tag="psT")
                    nc.tensor.transpose(psT[:], o2[:], ident_sb[:])
                    o2T = epip.tile([P, P], F16, tag="o2T")
                    nc.vector.tensor_copy(o2T[:], psT[:])
                    psH = psE.tile([P, fuse_cols], F32, tag="psH")
                    nc.tensor.matmul(psH[:], o2T[:], w2_sb[:],
                                     start=True, stop=True)
                    h2 = epip.tile([P, fuse_cols], F16, tag="h2")
                    nc.vector.tensor_copy(h2[:], psH[:])
                    nc.sync.dma_start(out=out[w * P:(w + 1) * P, :], in_=h2[:])
                else:
                    if elu:
                        res = epip.tile([P, HC], F32, tag="res")
                        nc.vector.tensor_tensor(out=res[:], in0=mx[:],
                                                in1=ex[:], op=OP.add)
                    else:
                        res = o1
                    nc.sync.dma_start(out=out[w * P:(w + 1) * P, :], in_=res[:])

            def edge_phase(_iv=None):
                psw_cur = None
                scount = [0]
                for g in range(n_groups):
                    tlo, thi = g * GRP, min(T, g * GRP + GRP)
                    ng = thi - tlo
                    xs_g = streamp.tile([P, GRP, C], F16, tag="xs")
                    nc.sync.dma_start(out=xs_g[:, :ng, :].rearrange(
                        "p g c -> p (g c)"),
                        in_=xs[:, tlo * C:thi * C])
                    # z = als + ald  [P, ng, heads] f32
                    zf = zp.tile([P, GRP, heads], F32, tag="zf")
                    z_eng = nc.gpsimd if CFG["z_add_pool"] else nc.vector
                    z_eng.tensor_tensor(
                        out=zf[:, :ng, :], in0=xs_g[:, :ng, HC:HC + heads],
                        in1=xs_g[:, :ng, HC + heads:HC + 2 * heads], op=OP.add)
                    nc.scalar.activation(zf[:, :ng, :], zf[:, :ng, :],
                                         AF.Prelu, alpha=NEG_SLOPE)
                    # exp expanded to all HC columns  [P, ng, HC] f16
                    expf = zp.tile([P, GRP, HC], F16, tag="expf")
                    zin = zf[:, :ng, :]
                    zin_b = bass.AP(zin.tensor, zin.offset,
                                    [zin.ap[0], [heads, ng], [1, heads],
                                     [0, hid]])
                    nc.scalar.activation(expf[:, :ng, :], zin_b, AF.Exp,
                                         bias=ebias_sb[:])
                    # rel column f16 -> f32 (tensor_scalar wants f32 scalars)
                    relf = zp.tile([P, GRP], F32, tag="relf")
                    nc.vector.tensor_copy(relf[:, :ng], xs_g[:, :ng, C - 1])
                    # S matrices, one per tile
                    S_g = sp_.tile([P, GRP, P], F16, tag="S")
                    pse = CFG["pool_s_every"]
                    for j in range(ng):
                        eng = nc.gpsimd if (
                            pse and scount[0] % pse == 0
                        ) else nc.vector
                        scount[0] += 1
                        eng.tensor_scalar(S_g[:, j, :], iota_sb[:],
                                          relf[:, j:j + 1], None,
                                          OP.is_equal)
                    # messages = h * exp  (all-SBUF packed f16)
                    msg_g = msgp.tile([P, GRP, HC], F16, tag="msg")
                    nc.vector.tensor_tensor(
                        out=msg_g[:, :ng, :], in0=xs_g[:, :ng, 0:HC],
                        in1=expf[:, :ng, :], op=OP.mult)
                    for j in range(ng):
                        t = tlo + j
                        w = tile_win[t]
                        if t == first_of_win[w]:
                            psw_cur = psW.tile([P, HC + heads], F32, tag="psW")
                        nc.tensor.matmul(
                            psw_cur[:, 0:HC], S_g[:, j, :], msg_g[:, j, :],
                            start=(t == first_of_win[w]), stop=False)
                        e_ap = expf[:, j, :]
                        e_str = bass.AP(e_ap.tensor, e_ap.offset,
                                        [e_ap.ap[0], [hid, heads]])
                        nc.tensor.matmul(
                            psw_cur[:, HC:HC + heads], S_g[:, j, :], e_str,
                            start=False, stop=(t == last_of_win[w]))
                        if t == last_of_win[w]:
                            epilogue(tile_win[t], psw_cur)

            if bench_loop > 1:
                with tc.For_i(0, bench_loop, 1) as _iv:
                    edge_phase(_iv)
            else:
                edge_phase()
    _finalize_kernel(nc)
    return nc


# ------------------------------------------------------------------ runner

def _fold_att(W, a):
    heads, hid = a.shape
    return np.einsum("ihc,hc->ih", W.reshape(W.shape[0], heads, hid), a)


class _GatRunner:
    def __init__(self, n_cores=N_CORES):
        self.C = n_cores
        self._graph = None
        self._graph_key = None
        self._kernels = {}

    def graph(self, edge_index, n_nodes):
        key = hash(np.asarray(edge_index).tobytes())
        if key != self._graph_key:
            self._graph = _Graph(edge_index, n_nodes, self.C)
            self._graph_key = key
            self._kernels.clear()
        return self._graph

    def node_kernel(self, g, c_in, c_out, bench_loop=1):
        key = ("N", g.T, c_in, c_out, bench_loop)
        if key not in self._kernels:
            self._kernels[key] = _build_node_kernel(g.wpc, c_in, c_out,
                                                    bench_loop)
        return self._kernels[key]

    def edge_kernel(self, name, g, heads, hid, elu, add_bias, fuse_cols,
                    bench_loop=1):
        key = (name, g.T, heads, hid, elu, add_bias, fuse_cols, bench_loop)
        if key not in self._kernels:
            self._kernels[key] = _build_edge_kernel(
                g.T, g.PC, g.wpc, heads, hid, elu, add_bias, fuse_cols,
                bench_loop)
        return self._kernels[key]

    @staticmethod
    def w1ext(W1, a_src1, a_dst1):
        return np.concatenate(
            [W1, _fold_att(W1, a_src1), _fold_att(W1, a_dst1)],
            axis=1).astype(np.float16)

    @staticmethod
    def w2ext(W2, a_src2, a_dst2):
        return np.concatenate(
            [W2, _fold_att(W2, a_src2), _fold_att(W2, a_dst2)],
            axis=1).astype(np.float16)

    def node_maps(self, g, x, wextv):
        xT_pad = np.zeros((x.shape[1], g.n_win * P), dtype=np.float16)
        xT_pad[:, :x.shape[0]] = np.asarray(x, np.float32).T
        return [{
            "xT": np.ascontiguousarray(
                xT_pad[:, k * g.shard_nodes:(k + 1) * g.shard_nodes]),
            "wext": wextv,
        } for k in range(self.C)]

    def edge_maps(self, g, table, heads, hid, fuse_w2=None, brep=None):
        HC = heads * hid
        iota_v = np.tile(np.arange(P, dtype=np.float16), (P, 1))
        maps = []
        for k in range(self.C):
            im = {
                "xs": g.build_stream(table, k, HC + heads, heads),
                "iota": iota_v,
            }
            if fuse_w2 is not None:
                im["ident"] = np.eye(P, dtype=np.float16)
                im["w2ext"] = fuse_w2
            if brep is not None:
                im["brep"] = brep
            maps.append(im)
        return maps

    def run(self, x, edge_index, W1, a_src1, a_dst1, b1, W2, a_src2, a_dst2,
            b2):
        C = self.C
        N, IN_C = x.shape
        HEADS, HID = a_src1.shape
        HC = HEADS * HID
        OUT_C = W2.shape[1]
        g = self.graph(edge_index, N)
        b1nz = bool(np.any(b1))
        b2nz = bool(np.any(b2))
        assert not b1nz and not b2nz, "nonzero biases not wired up"

        w1e = self.w1ext(W1, a_src1, a_dst1)          # [IN_C, HC+2*HEADS]
        w2e = self.w2ext(W2, a_src2, a_dst2)          # [HC, OUT_C+2]

        ncN = self.node_kernel(g, IN_C, w1e.shape[1])
        resN = run_bass_kernel_spmd(ncN, self.node_maps(g, x, w1e),
                                    core_ids=list(range(C)))
        table1 = np.concatenate([r["out"] for r in resN.results], axis=0)

        ncE1 = self.edge_kernel("E1", g, HEADS, HID, True, False,
                                w2e.shape[1])
        mapsE1 = self.edge_maps(g, table1, HEADS, HID, fuse_w2=w2e)
        resE1 = run_bass_kernel_spmd(ncE1, mapsE1, core_ids=list(range(C)))
        table2 = np.concatenate([r["out"] for r in resE1.results], axis=0)

        ncE2 = self.edge_kernel("E2", g, 1, OUT_C, False, False, 0)
        mapsE2 = self.edge_maps(g, table2, 1, OUT_C)
        resE2 = run_bass_kernel_spmd(ncE2, mapsE2, core_ids=list(range(C)))
        return np.concatenate([r["out"] for r in resE2.results], axis=0)[:N]


_RUNNER = _GatRunner()


def kernel(x, edge_index, W1, a_src1, a_dst1, b1, W2, a_src2, a_dst2, b2):
    """Full-input / full-output entry point. Returns [N, OUT_C] float32."""
    args = [np.asarray(v) for v in
            (x, edge_index, W1, a_src1, a_dst1, b1, W2, a_src2, a_dst2, b2)]
    return _RUNNER.run(*args).astype(np.float32)
